# revision 1
# baseline (speedup 1.0000x reference)
"""CrossInteraction kernel for TRN2, 8-core data parallel.

Math: interaction[b,i,j] = x1[b,i] * x2[b,j]
  mean_dim1[b,i] = x1[b,i] * mean_j(x2[b,j])
  mean_dim2[b,j] = x2[b,j] * mean_i(x1[b,i])
  out = concat([mean_dim1, mean_dim2], axis=1)   # (B, DIM1+DIM2)

The (B, DIM1, DIM2) interaction tensor is never materialized: per batch row
we need one row-mean of x1, one row-mean of x2, and two scaled copies.

Sharding: pure data parallel over batch - 256 rows / 8 cores = 32 rows/core.

Layout: each per-core tensor (32, F) is loaded as a [128, F/4] SBUF tile
with partition p = 32*c + b (c = feature-chunk 0..3, b = batch row), which
keeps every DVE op at full 128-lane width.

Critical-path design (vs the naive version):
- Loads: two plain DMACopies on the SP queue (HWDGE, lowest fixed costs),
  x2 first since its reduce is longest.
- Partition fold (sum the 4 chunk partials of each row and broadcast back
  to all 128 partitions) is ONE PE matmul per tensor with a precomputed
  0/1 mask S4[k,m] = (k===m mod 32), instead of a serial chain of shifted
  adds on DVE. S4 is built during the DMA-wait dead time from a gpsimd
  iota. Each matmul fires as soon as its reduce lands, so the s2 fold
  overlaps the x1 load latency.
- Stores: two dma_scatter_add's whose SWDGE descriptors are PREPARED on
  the Pool engine during the load dead time and fired with trigger_dma
  the moment the producing tensor_scalar finishes. This removes the
  HWDGE (625ns) + DGE-start (650ns) fixed costs from the store path.
  Output DRAM is pre-zeroed by the runner, so scatter-add == store.
  Scatter tokens are partition p = 32c+b carrying a contiguous slice of
  the output row b, addressed via an int16 index tensor ([128, 8], a
  16-row pattern replicated 8x) generated by iota + partition copies.
"""

import numpy as np

import concourse.bass as bass
import concourse.bacc as bacc
import concourse.tile as tile
from concourse import mybir
from concourse.instruction_name_ordered_set import InstructionNameOrderedSet
from concourse.bass_utils import run_bass_kernel_spmd

BATCH, DIM1, DIM2 = 256, 512, 1024
N_CORES = 8
B_LOC = BATCH // N_CORES  # 32 rows per core
F1 = DIM1 // 4  # 128
F2 = DIM2 // 4  # 256

_FP32 = mybir.dt.float32
_I32 = mybir.dt.int32
_I16 = mybir.dt.int16


def build_nc() -> bass.Bass:
    nc = bacc.Bacc(
        "TRN2",
        target_bir_lowering=False,
        debug=False,
        num_devices=N_CORES,
        num_swdge_queues=3,
    )
    x1 = nc.dram_tensor("x1", [B_LOC, DIM1], _FP32, kind="ExternalInput").ap()
    x2 = nc.dram_tensor("x2", [B_LOC, DIM2], _FP32, kind="ExternalInput").ap()
    out = nc.dram_tensor("out", [B_LOC, DIM1 + DIM2], _FP32, kind="ExternalOutput").ap()

    # DRAM views matching the [128, F/4] partition=32c+b SBUF layout.
    x1_v = x1.rearrange("b (c f) -> c b f", c=4)
    x2_v = x2.rearrange("b (c f) -> c b f", c=4)
    # Flat row views for the scatter stores: out row b splits into stride-F
    # pieces; token (b, c) of o1 lands at flat row 12b+c of the z=12 view,
    # token (b, c) of o2 at flat row 6b+2+c of the z=6 view.
    o1_flat = out.rearrange("b (z f) -> (b z) f", z=12)  # [384, 128]
    o2_flat = out.rearrange("b (z f) -> (b z) f", z=6)  # [192, 256]

    AL = mybir.AluOpType

    with tile.TileContext(nc) as tc:
        with (
            tc.tile_pool(name="p", bufs=1) as pool,
            tc.tile_pool(name="ps", bufs=1, space=bass.MemorySpace.PSUM) as psum,
        ):
            x1_t = pool.tile([128, F1], _FP32)
            x2_t = pool.tile([128, F2], _FP32)
            o1_t = pool.tile([128, 1, F1], _FP32)
            o2_t = pool.tile([128, 1, F2], _FP32)
            q = pool.tile([128, 2], _FP32)
            # Two separate PSUM tiles so each tensor_scalar only waits on its
            # own fold matmul (one shared tile coarsens the dep to both).
            s1 = psum.tile([128, 1], _FP32)
            s2 = psum.tile([128, 1], _FP32)
            mp = pool.tile([128, 128], _I32)
            mp2 = pool.tile([128, 128], _I32)
            s4 = pool.tile([128, 128], _FP32)
            pidx = pool.tile([128, 1], _I32)
            pmod = pool.tile([128, 1], _I32)
            pmodf = pool.tile([128, 1], _FP32)
            m12f = pool.tile([128, 1], _FP32)
            m6f = pool.tile([128, 1], _FP32)
            patA = pool.tile([128, 8], _I32)
            patB = pool.tile([128, 8], _I32)
            patAf = pool.tile([128, 8], _FP32)
            patBf = pool.tile([128, 8], _FP32)
            idxAf = pool.tile([128, 8], _FP32)
            idxBf = pool.tile([128, 8], _FP32)
            idxA = pool.tile([128, 8], _I16)
            idxB = pool.tile([128, 8], _I16)

            # ---- loads (SP queue, HWDGE): x2 first, its reduce is longest
            nc.sync.dma_start(x2_t[:], x2_v)
            nc.sync.dma_start(x1_t[:], x1_v)

            # ---- dead-time constant generation (gpsimd iotas) ----
            # Engine partition bases must be 32-aligned, so the idx tensors
            # ([128, 8] int16, a 16-row pattern replicated per 16 partitions,
            # value idx[p, 2c+h] = stride*(p%16) + pat(c, h)) are built
            # arithmetically over all 128 partitions:
            #   idx = pat_iota(columns) + (p & 15) * stride.
            # S4 fold mask source: mp[p, m] = m - p
            nc.gpsimd.iota(mp[:], [[1, 128]], base=0, channel_multiplier=-1)
            nc.gpsimd.iota(pidx[:], [[0, 1]], base=0, channel_multiplier=1)
            # idxA value for token p=32c+b (b=16h+q): 12b + c = 12q + (c+192h)
            nc.gpsimd.iota(patA[:], [[1, 4], [192, 2]], base=0, channel_multiplier=0)
            # idxB value: 6b + 2 + c = 6q + (c + 96h + 2)
            nc.gpsimd.iota(patB[:], [[1, 4], [96, 2]], base=2, channel_multiplier=0)

            # (DVE, dead time) idx = pat + (p & 15) * stride, computed in fp32
            # (AP-scalar add requires fp32 operands; HW forbids mixing bitwise
            # and arith ops in one tensor_scalar), cast to int16 at the end.
            nc.vector.tensor_scalar(pmod[:], pidx[:], 15, None, AL.bitwise_and)
            nc.vector.tensor_copy(pmodf[:], pmod[:])
            nc.vector.tensor_scalar(m12f[:], pmodf[:], 12.0, None, AL.mult)
            nc.vector.tensor_scalar(m6f[:], pmodf[:], 6.0, None, AL.mult)
            nc.vector.tensor_copy(patAf[:], patA[:])
            nc.vector.tensor_copy(patBf[:], patB[:])
            nc.vector.tensor_scalar(idxAf[:], patAf[:], m12f[:], None, AL.add)
            nc.vector.tensor_scalar(idxBf[:], patBf[:], m6f[:], None, AL.add)
            nc.vector.tensor_copy(idxA[:], idxAf[:])
            nc.vector.tensor_copy(idxB[:], idxBf[:])

            # S4[k, m] = 1.0 where (m - k) % 32 == 0 else 0.0 (DVE, dead time)
            nc.vector.tensor_scalar(mp2[:], mp[:], 31, None, AL.bitwise_and)
            nc.vector.tensor_scalar(s4[:], mp2[:], 0, None, AL.is_equal)

            # ---- store descriptor prep (Pool/SWDGE, dead time) ----
            # The DMA-completion sem baked into each prepared descriptor must
            # be the Tile DMASW lane sem (assigned round-robin in scheduled
            # order), or the end-of-context waits on DMASW{n} never fire.
            swdge_sems = tc.sems.swdge_block()
            n128 = nc.gpsimd.to_reg(128)
            prep_a = nc.gpsimd.dma_scatter_add(
                o1_flat,
                o1_t[:],
                idxA[:],
                128,
                n128,
                F1,
                prepare_only=True,
                sem=swdge_sems[0],
                queue_num=1,
            )
            prep_b = nc.gpsimd.dma_scatter_add(
                o2_flat,
                o2_t[:],
                idxB[:],
                128,
                n128,
                F2,
                prepare_only=True,
                sem=swdge_sems[1],
                queue_num=2,
            )

            # ---- compute ----
            # Per-chunk row partials: q[:,1] = x2 partials, q[:,0] = x1 partials
            nc.vector.reduce_sum(q[:, 1:2], x2_t[:], axis=mybir.AxisListType.X)
            nc.vector.reduce_sum(q[:, 0:1], x1_t[:], axis=mybir.AxisListType.X)

            # Fold+broadcast via PE: s[p] = sum_{k=p mod 32} q[k].
            # Two matmuls so the x2 fold fires before the x1 load lands.
            nc.tensor.matmul(s2[:], s4[:], q[:, 1:2])
            nc.tensor.matmul(s1[:], s4[:], q[:, 0:1])

            # o1 = x1 * mean(x2) ; o2 = x2 * mean(x1)  (scalars read from PSUM)
            nc.vector.tensor_scalar(
                o1_t[:, 0, :], x1_t[:], s2[:], 1.0 / DIM2, AL.mult, AL.mult
            )
            nc.vector.tensor_scalar(
                o2_t[:, 0, :], x2_t[:], s1[:], 1.0 / DIM1, AL.mult, AL.mult
            )

            # ---- fire the prepared stores ----
            # nosync chain prep_b -> trig_a -> trig_b pins the Pool SEQ order:
            # without it Tile schedules prep_b AFTER trig_a's ts_o1 wait,
            # pushing the second store's desc-gen onto the critical path.
            trig_a = nc.gpsimd.trigger_dma(count=None, queue_num=1)
            dep_a = InstructionNameOrderedSet()
            dep_a.add(prep_b.ins.name)
            trig_a.ins.add_nosync_dependencies_from(dep_a)
            trig_b = nc.gpsimd.trigger_dma(count=None, queue_num=2)
            dep_b = InstructionNameOrderedSet()
            dep_b.add(trig_a.ins.name)
            trig_b.ins.add_nosync_dependencies_from(dep_b)
            # Tile adds a WAW sync edge trig_b -> prep_a (both scatters write
            # `out`), which would stall store B until store A's DMA-completion
            # semaphore (~1.1us). The two scatters write disjoint byte ranges
            # of `out`, so the hazard is spurious — drop it.
            trig_b.ins.try_remove_dependency(prep_a.ins.name)
    nc.compile()
    return nc


def run(x1: np.ndarray, x2: np.ndarray, trace: bool = False):
    """Build + run on 8 cores; returns (full_output, BassKernelResults)."""
    nc = build_nc()
    x1 = np.ascontiguousarray(np.asarray(x1, dtype=np.float32))
    x2 = np.ascontiguousarray(np.asarray(x2, dtype=np.float32))
    in_maps = [
        {
            "x1": x1[i * B_LOC:(i + 1) * B_LOC],
            "x2": x2[i * B_LOC:(i + 1) * B_LOC],
        }
        for i in range(N_CORES)
    ]
    res = run_bass_kernel_spmd(nc, in_maps, list(range(N_CORES)), trace=trace)
    full = np.concatenate([r["out"] for r in res.results], axis=0)
    return full, res


def kernel(x1: np.ndarray, x2: np.ndarray) -> np.ndarray:
    full, _ = run(x1, x2, trace=False)
    return full

